# revision 1
# baseline (speedup 1.0000x reference)
"""Distributed GQA attention kernel for 8 TRN2 NeuronCores.

Problem: B=2, S=2048, D=2048, 32 q-heads / 8 kv-heads, hd=64, causal + RoPE.

Strategy (sequence-sharded "context parallel"):
  - Each core owns 2 zigzag row-blocks per batch (blocks i and 15-i of 16),
    512 rows total. It computes Q for all 32 heads on its rows, K/V for all
    8 kv-heads on its rows, applies RoPE, then AllGathers K/V (about 1MB/rank,
    far cheaper than the 33MB AllReduce a head-sharded split would need).
  - Attention runs fully "transposed": projections produce qT/kT (head-dim on
    partitions) directly from x^T (host-pretransposed), scoresT = kT_tile.T @ qT
    come out with keys on partitions, probsT feeds P@V as the moving operand with
    V in natural layout as the stationary operand, and the PV output outT
    [hd, rows] is exactly the lhsT layout the output projection needs.
    No on-device transposes anywhere.
  - Softmax without max-subtraction (scores are bounded ~|4| for this data):
    probs = exp(s/8) * exp(mask), with the additive mask converted host-side to
    multiplicative per-tile factors (1/0 for causal). The denominator comes free
    from a ones-column appended to V (M=65 PV matmuls); normalization is applied
    to the attention output with a K=2 broadcast matmul + elementwise multiply.
  - Weight matrices are permuted host-side so that (a) RoPE's (even,odd) pairs
    are de-interleaved into [a(32)|b(32)] partition halves (RoPE becomes 3
    elementwise ops + partition-swap DMAs) and (b) q-heads pair up so 2 GQA
    groups pack the 128x128 PE array (K=64 row-group packing) in one shot.
  - Matmuls run in bf16 (1 cycle/row vs fp32's 4); psums/softmax stay fp32.

kernel(**inputs) -> np.ndarray  takes full inputs, returns full [2,2048,2048].
"""

import functools
import os
import sys
import types

import numpy as np
import ml_dtypes


class _StageDone(Exception):
    pass

BF16 = ml_dtypes.bfloat16

B, S, D = 2, 2048, 2048
NH, NKV, HD = 32, 8, 64
NREP = NH // NKV
NCORES = 8
BLK = 128
NBLK = S // BLK          # 16 blocks per batch
RPB = 2 * BLK            # rows per core per batch (2 blocks)
RT = B * RPB             # rows per core total = 512
KD = NKV * HD            # 512
VROW = 2 * HD + 2        # 130: [v_a | 1 | v_b | 1] per kv pair
CONTRIB_W = 4 * VROW     # 520


def _heads_of_tile(t):
    gg, m = divmod(t, 4)
    return 8 * gg + m, 8 * gg + 4 + m


def _core_blocks(i):
    return i, NBLK - 1 - i


# --------------------------------------------------------------------------
# device graph
# --------------------------------------------------------------------------

@functools.lru_cache(maxsize=None)
def _build_nc():
    import concourse.bacc as bacc
    import concourse.mybir as mybir
    import concourse.tile as tile

    BF = mybir.dt.bfloat16
    F32 = mybir.dt.float32
    EXP = mybir.ActivationFunctionType.Exp

    nc = bacc.Bacc(trn_type="TRN2", target_bir_lowering=False, debug=False,
                   num_devices=NCORES)

    xT_d = nc.declare_dram_parameter("xT", [D, RT], BF, isOutput=False)
    wq_d = nc.declare_dram_parameter("wq", [16, 16, 128, 128], BF, isOutput=False)
    wk_d = nc.declare_dram_parameter("wk", [16, 4, 128, 128], BF, isOutput=False)
    wv_d = nc.declare_dram_parameter("wv", [D, KD], BF, isOutput=False)
    wo_d = nc.declare_dram_parameter("wo", [D, D], BF, isOutput=False)
    crep_d = nc.declare_dram_parameter("crep", [128, RT], BF, isOutput=False)
    ssign_d = nc.declare_dram_parameter("ssign", [128, RT], BF, isOutput=False)
    mask_d = nc.declare_dram_parameter("maskm", [NBLK, 128, 512], BF, isOutput=False)
    out_d = nc.declare_dram_parameter("out", [RT, D], F32, isOutput=True)

    with tile.TileContext(nc) as tc:
        with tc.tile_pool(name="dram", bufs=1, space="DRAM") as dpool, \
             tc.tile_pool(name="const", bufs=1) as cpool, \
             tc.tile_pool(name="persist", bufs=1) as ppool, \
             tc.tile_pool(name="wstream", bufs=6) as wpool, \
             tc.tile_pool(name="work", bufs=3) as tpool, \
             tc.tile_pool(name="attn", bufs=3) as apool, \
             tc.tile_pool(name="ps", bufs=1, space="PSUM") as pspool:

            contrib = dpool.tile([2 * KD, CONTRIB_W], BF, name="contrib")
            gathered = dpool.tile([NCORES * 2 * KD, CONTRIB_W], BF,
                                  name="gathered", addr_space="Shared")

            # ---- constants ----
            crep = cpool.tile([128, RT], BF, name="crep", tag="crep")
            nc.sync.dma_start(out=crep[:, :], in_=crep_d[:, :])
            ssign = cpool.tile([128, RT], BF, name="ssign", tag="ssign")
            nc.sync.dma_start(out=ssign[:, :], in_=ssign_d[:, :])
            zt = cpool.tile([128, 512], BF, name="zt", tag="zt")
            nc.gpsimd.memset(zt[:, :], 0.0)
            msk = []
            for kb in range(NBLK):
                mt = cpool.tile([128, 512], BF, name=f"msk{kb}", tag=f"msk{kb}")
                nc.sync.dma_start(out=mt[:, :], in_=mask_d[kb, :, :])
                msk.append(mt)

            # ---- xT resident ----
            xt = []
            for k in range(16):
                t_ = ppool.tile([128, RT], BF, name=f"xt{k}", tag=f"xt{k}")
                nc.sync.dma_start(out=t_[:, :], in_=xT_d[k * 128:(k + 1) * 128, :])
                xt.append(t_)

            def rope(raw, out_t, out_halves=None):
                """raw [128, RT] bf16 (layout [a|b|a|b] x32) -> rotated+mixed.
                out_halves: optional pair of [64, RT] tiles to receive the two
                head halves at partition base 0 (avoids base-64 matmul operands,
                which fault the runtime)."""
                rot = tpool.tile([128, RT], BF, name="rot", tag="rot")
                for (db, sb) in ((0, 32), (32, 0), (64, 96), (96, 64)):
                    nc.gpsimd.dma_start(out=rot[db:db + 32, :],
                                        in_=raw[sb:sb + 32, :])
                t2 = tpool.tile([128, RT], BF, name="ropea", tag="ropea")
                t3 = tpool.tile([128, RT], BF, name="ropeb", tag="ropeb")
                nc.vector.tensor_mul(t2[:, :], raw[:, :], crep[:, :])
                nc.vector.tensor_mul(t3[:, :], rot[:, :], ssign[:, :])
                if out_halves is None:
                    nc.vector.tensor_add(out_t[:, :], t2[:, :], t3[:, :])
                else:
                    ha, hb = out_halves
                    nc.vector.tensor_add(ha[0:64, :], t2[0:64, :], t3[0:64, :])
                    nc.vector.tensor_add(hb[0:64, :], t2[64:128, :], t3[64:128, :])

            # ---- K projection + RoPE -> contrib ----
            kT = []
            for g in range(4):
                ps = pspool.tile([128, RT], F32, name=f"psk{g}", tag=f"pv{g % 4}")
                for kt in range(16):
                    wkt = wpool.tile([128, 128], BF, name="wkt", tag="wk")
                    (nc.sync if kt % 2 == 0 else nc.gpsimd).dma_start(
                        out=wkt[:, :], in_=wk_d[kt, g, :, :])
                    nc.tensor.matmul(ps[:, :], lhsT=wkt[:, :], rhs=xt[kt][:, :],
                                     start=(kt == 0), stop=(kt == 15))
                kraw = tpool.tile([128, RT], BF, name="kraw", tag="kraw")
                nc.vector.tensor_copy(out=kraw[:, :], in_=ps[:, :])
                kt_t = tpool.tile([128, RT], BF, name=f"kT{g}", tag="kTout")
                rope(kraw, kt_t)
                kT.append(kt_t)
                nc.sync.dma_start(out=contrib[g * 128:(g + 1) * 128, 0:RT],
                                  in_=kt_t[:, :])

            # ---- V projection -> contrib (with ones columns) ----
            for r in range(4):
                ps = pspool.tile([128, KD], F32, name=f"psv{r}", tag=f"pv{r % 4}")
                for kt in range(16):
                    wvt = wpool.tile([128, KD], BF, name="wvt", tag="wv")
                    (nc.sync if kt % 2 == 0 else nc.gpsimd).dma_start(
                        out=wvt[:, :], in_=wv_d[kt * 128:(kt + 1) * 128, :])
                    nc.tensor.matmul(ps[:, :], lhsT=xt[kt][:, r * 128:(r + 1) * 128],
                                     rhs=wvt[:, :], start=(kt == 0), stop=(kt == 15))
                vsb = tpool.tile([128, CONTRIB_W], BF, name="vsb", tag="vsb")
                vdst = vsb.rearrange("p (g t u) -> p g t u", g=4, t=2, u=VROW // 2)
                vsrc = ps.rearrange("p (g t u) -> p g t u", g=4, t=2, u=HD)
                nc.scalar.copy(out=vdst[:, :, :, 0:HD], in_=vsrc[:, :, :, :])
                nc.gpsimd.memset(vdst[:, :, :, HD:HD + 1], 1.0)
                nc.sync.dma_start(
                    out=contrib[KD + r * 128:KD + (r + 1) * 128, :],
                    in_=vsb[:, :])

            # ---- AllGather K/V ----
            nc.gpsimd.collective_compute(
                "AllGather", mybir.AluOpType.bypass,
                replica_groups=[list(range(NCORES))],
                ins=[contrib[:, :].opt()], outs=[gathered[:, :].opt()],
            )

            # ---- Q projection + RoPE (overlaps the AllGather) ----
            # qpa/qpb[gg][p]: [64, 1024] = cols [b0: m=2p | m=2p+1, b1: same],
            # a/b = first/second head of the GQA pair (kv 2gg / 2gg+1).
            qpa = [[None, None] for _ in range(4)]
            qpb = [[None, None] for _ in range(4)]
            for gg in range(4):
                for p in range(2):
                    qpa[gg][p] = ppool.tile([64, 1024], BF, name=f"qpa{gg}{p}",
                                            tag=f"qpa{gg}{p}")
                    qpb[gg][p] = ppool.tile([64, 1024], BF, name=f"qpb{gg}{p}",
                                            tag=f"qpb{gg}{p}")
            for t in range(16):
                gg, m = divmod(t, 4)
                p, half = divmod(m, 2)
                ps = pspool.tile([128, RT], F32, name=f"psq{t}", tag=f"pv{t % 4}")
                for kt in range(16):
                    wqt = wpool.tile([128, 128], BF, name="wqt", tag="wq")
                    (nc.sync if kt % 2 == 0 else nc.gpsimd).dma_start(
                        out=wqt[:, :], in_=wq_d[kt, t, :, :])
                    nc.tensor.matmul(ps[:, :], lhsT=wqt[:, :], rhs=xt[kt][:, :],
                                     start=(kt == 0), stop=(kt == 15))
                qraw = tpool.tile([128, RT], BF, name="qraw", tag="qraw")
                nc.vector.tensor_copy(out=qraw[:, :], in_=ps[:, :])
                rot = tpool.tile([128, RT], BF, name="rot", tag="rot")
                for (db, sb) in ((0, 32), (32, 0), (64, 96), (96, 64)):
                    nc.gpsimd.dma_start(out=rot[db:db + 32, :],
                                        in_=qraw[sb:sb + 32, :])
                t2 = tpool.tile([128, RT], BF, name="ropea", tag="ropea")
                t3 = tpool.tile([128, RT], BF, name="ropeb", tag="ropeb")
                nc.vector.tensor_mul(t2[:, :], qraw[:, :], crep[:, :])
                nc.vector.tensor_mul(t3[:, :], rot[:, :], ssign[:, :])
                for b_ in range(2):
                    d0 = b_ * 512 + half * 256
                    s0 = b_ * 256
                    nc.vector.tensor_add(qpa[gg][p][0:64, d0:d0 + 256],
                                         t2[0:64, s0:s0 + 256],
                                         t3[0:64, s0:s0 + 256])
                    nc.vector.tensor_add(qpb[gg][p][0:64, d0:d0 + 256],
                                         t2[64:128, s0:s0 + 256],
                                         t3[64:128, s0:s0 + 256])

            # ---- attention ----
            attnT = []
            for t in range(16):
                at = ppool.tile([128, RT], BF, name=f"attnT{t}", tag=f"attnT{t}")
                attnT.append(at)

            KEYS = (("a", 0), ("a", 1), ("b", 0), ("b", 1))
            for b in range(B):
                for gg in range(4):
                    pv = {}
                    for i_, key in enumerate(KEYS):
                        pv[key] = pspool.tile([65, 512], F32,
                                              name=f"pvb{i_}", tag=f"pv{i_}")
                    pending = []
                    for kb in range(NBLK):
                        r = kb if kb < 8 else 15 - kb
                        sslot = 0 if kb < 8 else 1
                        kof = b * RPB + sslot * 128
                        ksl_a = apool.tile([64, 128], BF, name="ksla", tag="ksla", bufs=6)
                        nc.sync.dma_start(
                            out=ksl_a[:, :],
                            in_=gathered[1024 * r + 128 * gg:
                                         1024 * r + 128 * gg + 64,
                                         kof:kof + 128])
                        ksl_b = apool.tile([64, 128], BF, name="kslb", tag="kslb", bufs=6)
                        nc.gpsimd.dma_start(
                            out=ksl_b[:, :],
                            in_=gathered[1024 * r + 128 * gg + 64:
                                         1024 * r + 128 * (gg + 1),
                                         kof:kof + 128])
                        vsl = apool.tile([128, VROW], BF, name="vsl", tag="vsl", bufs=8)
                        nc.sync.dma_start(
                            out=vsl[:, :],
                            in_=gathered[1024 * r + KD + kof:
                                         1024 * r + KD + kof + 128,
                                         VROW * gg:VROW * (gg + 1)])
                        cur = []
                        for half, ksl, qgrp, vcol in (
                                ("a", ksl_a, qpa[gg], 0),
                                ("b", ksl_b, qpb[gg], 65)):
                            for p in range(2):
                                sc = pspool.tile([128, 512], F32, name="sc",
                                                 tag="sc", bufs=4)
                                nc.tensor.matmul(
                                    sc[:, :], lhsT=ksl[:, :],
                                    rhs=qgrp[p][0:64, b * 512:b * 512 + 512],
                                    start=True, stop=True)
                                probs2 = apool.tile([128, 512], BF, name="probs2",
                                                    tag="probs2", bufs=10)
                                nc.scalar.activation(out=probs2[:, :], in_=sc[:, :],
                                                     func=EXP, scale=0.125)
                                pam2 = apool.tile([128, 512], BF, name="pam2",
                                                  tag="pam2", bufs=14)
                                nc.vector.tensor_mul(pam2[:, :], probs2[:, :],
                                                     msk[kb][:, :])
                                cur.append((half, p, vcol, pam2))
                        # PV matmuls run two kbs behind the scores so the PE
                        # never stalls on the exp/mask round-trip and the ACT
                        # always has a backlog of score tiles to exp.
                        pending.append((kb, vsl, cur))
                        if len(pending) > 3:
                            pkb, pvsl, plist = pending.pop(0)
                            for (half, p, vcol, pam2) in plist:
                                nc.tensor.matmul(
                                    pv[(half, p)][0:65, :],
                                    lhsT=pvsl[:, vcol:vcol + 65], rhs=pam2[:, :],
                                    start=(pkb == 0), stop=False)
                    for (pkb, pvsl, plist) in pending:
                        for (half, p, vcol, pam2) in plist:
                            nc.tensor.matmul(
                                pv[(half, p)][0:65, :],
                                lhsT=pvsl[:, vcol:vcol + 65], rhs=pam2[:, :],
                                start=(pkb == 0), stop=(pkb == NBLK - 1))

                    # ---- normalization ----
                    sums4 = apool.tile([128, 512], F32, name="sums4",
                                       tag="sums4", bufs=2)
                    for i_, key in enumerate(KEYS):
                        nc.vector.tensor_copy(out=sums4[32 * i_:32 * i_ + 1, :],
                                              in_=pv[key][64:65, :])
                    rec4 = apool.tile([128, 512], F32, name="rec4",
                                      tag="rec4", bufs=2)
                    nc.vector.reciprocal(out=rec4[:, :], in_=sums4[:, :])
                    for i_, (half, p) in enumerate(KEYS):
                        rec2 = apool.tile([1, 512], F32, name="rec2",
                                          tag="rec2", bufs=2)
                        # partition_broadcast reads physical partition 0 of its
                        # source tile (AP partition offsets are ignored), so
                        # stage each head-pair's row into a row-0 tile first.
                        nc.vector.tensor_copy(out=rec2[0:1, :],
                                              in_=rec4[32 * i_:32 * i_ + 1, :])
                        rep = apool.tile([128, 512], F32, name="repbc",
                                         tag="repbc", bufs=2)
                        nc.gpsimd.partition_broadcast(rep[:, :], rec2[0:1, :])
                        for mh in range(2):
                            t = 4 * gg + 2 * p + mh
                            qs = mh * 256
                            if half == "a":
                                nc.vector.tensor_mul(
                                    attnT[t][0:64, b * RPB:b * RPB + 256],
                                    pv[(half, p)][0:64, qs:qs + 256],
                                    rep[0:64, qs:qs + 256])
                            else:
                                nc.vector.tensor_mul(
                                    attnT[t][64:128, b * RPB:b * RPB + 256],
                                    pv[(half, p)][0:64, qs:qs + 256],
                                    rep[64:128, qs:qs + 256])


            # ---- output projection ----
            for dc in range(4):
                po = [pspool.tile([128, 512], F32, name=f"po{rt}", tag=f"pv{rt}")
                      for rt in range(4)]
                for t in range(16):
                    wot = wpool.tile([128, 512], BF, name="wot", tag="wo")
                    (nc.sync if t % 2 == 0 else nc.gpsimd).dma_start(
                        out=wot[:, :],
                        in_=wo_d[t * 128:(t + 1) * 128, dc * 512:(dc + 1) * 512])
                    for rt in range(4):
                        nc.tensor.matmul(po[rt][:, :],
                                         lhsT=attnT[t][:, rt * 128:(rt + 1) * 128],
                                         rhs=wot[:, :],
                                         start=(t == 0), stop=(t == 15))
                for rt in range(4):
                    ob = apool.tile([128, 512], F32, name="ob", tag="ob")
                    nc.vector.tensor_copy(out=ob[:, :], in_=po[rt][:, :])
                    nc.sync.dma_start(
                        out=out_d[rt * 128:(rt + 1) * 128,
                                  dc * 512:(dc + 1) * 512],
                        in_=ob[:, :])

    nc.compile()
    return nc


# --------------------------------------------------------------------------
# host-side sharding / layout prep
# --------------------------------------------------------------------------

def _prep_shared(wq, wk, wv, wo):
    qcol = np.zeros(D, np.int64)
    worow = np.zeros(D, np.int64)
    for t in range(16):
        ha, hb = _heads_of_tile(t)
        for half, h in enumerate((ha, hb)):
            base = t * 128 + half * 64
            qcol[base:base + 32] = h * 64 + np.arange(0, 64, 2)
            qcol[base + 32:base + 64] = h * 64 + np.arange(1, 64, 2)
            worow[base:base + 64] = h * 64 + np.arange(64)
    kcol = np.zeros(KD, np.int64)
    for g in range(NKV):
        base = g * 64
        kcol[base:base + 32] = g * 64 + np.arange(0, 64, 2)
        kcol[base + 32:base + 64] = g * 64 + np.arange(1, 64, 2)

    wq_t = wq[:, qcol].reshape(16, 128, 16, 128).transpose(0, 2, 1, 3)
    wq_t = np.ascontiguousarray(wq_t).astype(BF16)
    wk_t = wk[:, kcol].reshape(16, 128, 4, 128).transpose(0, 2, 1, 3)
    wk_t = np.ascontiguousarray(wk_t).astype(BF16)
    wv_c = np.ascontiguousarray(wv).astype(BF16)
    wo_c = np.ascontiguousarray(wo[worow, :]).astype(BF16)
    return wq_t, wk_t, wv_c, wo_c


def _prep_core(i, x, freqs_cos, freqs_sin, mask):
    bi, bj = _core_blocks(i)
    rows = np.concatenate([np.arange(bi * BLK, (bi + 1) * BLK),
                           np.arange(bj * BLK, (bj + 1) * BLK)])
    xs = np.concatenate([x[0, rows, :], x[1, rows, :]], axis=0)       # [512, D]
    xT = np.ascontiguousarray(xs.T).astype(BF16)                      # [D, 512]

    posf = np.concatenate([rows, rows])                               # [512]
    j = np.arange(128) % 32
    crep = freqs_cos[posf][:, j].T.astype(BF16)                       # [128, 512]
    sgn = np.where((np.arange(128) // 32) % 2 == 0, -1.0, 1.0).astype(np.float32)
    ssign = (freqs_sin[posf][:, j].T * sgn[:, None]).astype(BF16)

    maskm = np.zeros((NBLK, 128, 256), np.float32)
    for kb in range(NBLK):
        krows = mask[:, kb * BLK:(kb + 1) * BLK]                      # [S, 128]
        for col, blkq in enumerate((bi, bj)):
            madd = krows[blkq * BLK:(blkq + 1) * BLK, :]              # [128q,128k]
            maskm[kb][:, col * 128:(col + 1) * 128] = np.exp(madd.T)
    maskm = np.tile(maskm, (1, 1, 2)).astype(BF16)
    return xT, crep, ssign, maskm


def _assemble(results):
    out = np.empty((B, S, D), np.float32)
    for i in range(NCORES):
        bi, bj = _core_blocks(i)
        r = results[i]["out"]
        out[0, bi * BLK:(bi + 1) * BLK] = r[0:128]
        out[0, bj * BLK:(bj + 1) * BLK] = r[128:256]
        out[1, bi * BLK:(bi + 1) * BLK] = r[256:384]
        out[1, bj * BLK:(bj + 1) * BLK] = r[384:512]
    return out


LAST_RUN_INFO = {}


def kernel(x, freqs_cos, freqs_sin, mask, wq, wk, wv, wo, start_pos=0):
    from concourse.bass_utils import run_bass_kernel_spmd

    x = np.asarray(x, dtype=np.float32)
    freqs_cos = np.asarray(freqs_cos, dtype=np.float32)
    freqs_sin = np.asarray(freqs_sin, dtype=np.float32)
    mask = np.asarray(mask, dtype=np.float32)
    wq = np.asarray(wq, dtype=np.float32)
    wk = np.asarray(wk, dtype=np.float32)
    wv = np.asarray(wv, dtype=np.float32)
    wo = np.asarray(wo, dtype=np.float32)

    wq_t, wk_t, wv_c, wo_c = _prep_shared(wq, wk, wv, wo)
    in_maps = []
    for i in range(NCORES):
        xT, crep, ssign, maskm = _prep_core(i, x, freqs_cos, freqs_sin, mask)
        in_maps.append({
            "xT": xT, "wq": wq_t, "wk": wk_t, "wv": wv_c, "wo": wo_c,
            "crep": crep, "ssign": ssign, "maskm": maskm,
        })

    nc = _build_nc()

    trace = bool(int(os.environ.get("KERNEL_TRACE", "0")))
    kwargs = {}
    if trace:
        _install_ntff_hook()
        import concourse.bass_utils as bass_utils
        bass_utils.upload_artifacts = lambda tmpdir: tmpdir
        import tempfile
        tmpdir = tempfile.mkdtemp(prefix="attn_trace_")
        kwargs = {"trace": True, "tmpdir": tmpdir}

    res = run_bass_kernel_spmd(nc, in_maps, core_ids=list(range(NCORES)),
                               **kwargs)
    LAST_RUN_INFO.clear()
    LAST_RUN_INFO.update({
        "exec_time_ns": res.exec_time_ns,
        "tmpdir": kwargs.get("tmpdir"),
        "res": res,
    })
    return _assemble(res.results)


def _install_ntff_hook():
    if "antenv.axon_hooks" not in sys.modules:
        import antenv

        mod = types.ModuleType("antenv.axon_hooks")
        mod._hook = None
        mod.set_axon_ntff_profile_hook = lambda h: setattr(mod, "_hook", h)
        mod.get_axon_ntff_profile_hook = lambda: mod._hook
        sys.modules["antenv.axon_hooks"] = mod
        antenv.axon_hooks = mod
    from trn_agent_boot.trn_boot import _ntff_profile_via_ctypes
    from antenv.axon_hooks import set_axon_ntff_profile_hook as _set

    _set(_ntff_profile_via_ctypes("/opt/axon/libaxon_pjrt.so"))

